# revision 10
# baseline (speedup 1.0000x reference)
"""Trainium2 Bass kernel for per-head causal attention (nn_Attention_52896817217709).

Sharding: 8 cores = 4 head-groups (3 heads each) x 2 batches.
Per core, per head h (S=2048, D_MODEL=768, D_HEAD=64):
  qT/kT/vT = W^T @ X^T  with chunk-pairs packed on the two PE column halves
  (tile_position (0,0)/(0,64)) so each projection costs ~6144 PE cycles.
  qT/kT duplicated across both partition halves (SBUF->SBUF DMA) so the
  scores matmuls can be 2-way row-packed (K=64 halves, concurrent).
  Causal diag-tile masking is an additive PE matmul (identity^T @ maskU).
  Attention runs in two passes over chunk pairs (q 0:1024, then 1024:2048)
  with [128,1024] two-bank PSUM score tiles -> one exp per k-tile.
  vp (PV lhsT, [k,d|1]) built by DMA-transpose from checkerboarded vT.
  z' accumulates [65, 512] per chunk (row 64 = softmax sums via ones col).
  out = (z'^T_j @ [W_O; b_O/H]) * rc_j  with rc = 1/sums; evac on DVE.
Output stored fp16; exp is the only scalar-engine work.
"""
import sys
import os
import numpy as np

for _p in ("/opt/trn_rl_repo", "/root/.axon_site/_ro/trn_rl_repo"):
    if os.path.isdir(_p) and _p not in sys.path:
        sys.path.insert(0, _p)

import concourse.bass as bass
import concourse.tile as tile
from concourse import bacc, mybir
from concourse.bass_utils import run_bass_kernel_spmd

F32 = mybir.dt.float32
FP16 = mybir.dt.float16
AF = mybir.ActivationFunctionType

B, S, H, DM, DH = 2, 2048, 12, 768, 64
HPC = 3            # heads per core
NT = S // 128      # 16 k-tiles
MT = DM // 128     # 6 m-tiles
N_CORES = 8
NEG = -60000.0     # additive causal-mask constant (fp16-safe)
SCALE = 0.125      # 1/sqrt(DH)


def build_program(debug=False):
    nc = bacc.Bacc("TRN2", target_bir_lowering=False, debug=False)

    xq = nc.dram_tensor("xq", [HPC, DM, S], FP16, kind="ExternalInput")
    xk = nc.dram_tensor("xk", [HPC, DM, S], FP16, kind="ExternalInput")
    xv = nc.dram_tensor("xv", [HPC, DM, S], FP16, kind="ExternalInput")
    wq = nc.dram_tensor("wq", [HPC, MT, 128, DH], FP16, kind="ExternalInput")
    wk = nc.dram_tensor("wk", [HPC, MT, 128, DH], FP16, kind="ExternalInput")
    wv = nc.dram_tensor("wv", [HPC, MT, 128, DH], FP16, kind="ExternalInput")
    wo = nc.dram_tensor("wo", [HPC, 128, DM], FP16, kind="ExternalInput")
    bq = nc.dram_tensor("bq", [HPC, 128, 1], F32, kind="ExternalInput")
    bk = nc.dram_tensor("bk", [HPC, 128, 1], F32, kind="ExternalInput")
    bv = nc.dram_tensor("bv", [HPC, 128, 1], F32, kind="ExternalInput")
    identh = nc.dram_tensor("identh", [128, 128], FP16, kind="ExternalInput")
    masku = nc.dram_tensor("masku", [128, 128], FP16, kind="ExternalInput")
    out = nc.dram_tensor("out", [HPC, S, DM], FP16, kind="ExternalOutput")
    if debug:
        dqT = nc.dram_tensor("dqT", [128, S], FP16, kind="ExternalOutput")
        dkT = nc.dram_tensor("dkT", [128, S], FP16, kind="ExternalOutput")
        dvT = nc.dram_tensor("dvT", [128, S], FP16, kind="ExternalOutput")
        dvp = nc.dram_tensor("dvp", [128, NT, DH], FP16,
                             kind="ExternalOutput")
        dzT = nc.dram_tensor("dzT", [128, S], FP16, kind="ExternalOutput")
        drc = nc.dram_tensor("drc", [128, NT], F32, kind="ExternalOutput")

    TEN = {"q": (xq, wq, bq), "k": (xk, wk, bk), "v": (xv, wv, bv)}

    with tile.TileContext(nc) as tc:
        with (
            tc.tile_pool(name="wpool", bufs=1) as wpool,
            tc.tile_pool(name="xp", bufs=8) as x_pool,
            tc.tile_pool(name="wt", bufs=2) as wt_pool,
            tc.tile_pool(name="qk", bufs=2) as qk_pool,
            tc.tile_pool(name="vp", bufs=24) as vp_pool,
            tc.tile_pool(name="pp", bufs=6) as p_pool,
            tc.tile_pool(name="zt", bufs=2) as zt_pool,
            tc.tile_pool(name="rc", bufs=2) as rc_pool,
            tc.tile_pool(name="ob", bufs=3) as ob_pool,
            tc.tile_pool(name="psa", bufs=2, space="PSUM") as ps_aux,
            tc.tile_pool(name="pss", bufs=2, space="PSUM") as ps_s,
            tc.tile_pool(name="psz", bufs=2, space="PSUM") as ps_z,
        ):
            id_sb = wpool.tile([128, 128], FP16, name="id_sb")
            nc.gpsimd.dma_start(id_sb[:], identh[:])
            mask_sb = wpool.tile([128, 128], FP16, name="mask_sb")
            nc.gpsimd.dma_start(mask_sb[:], masku[:])
            ones_sb = wpool.tile([128, 1], FP16, name="ones_sb")
            nc.gpsimd.memset(ones_sb[:], 1.0)

            st = [dict() for _ in range(HPC)]

            def emit_loads(h):
                """X halves on sync (q,v) / scalar (k) rings; weights gpsimd."""
                for t, ring in (("q", nc.sync), ("k", nc.scalar),
                                ("v", nc.sync)):
                    xd, wd, bd = TEN[t]
                    halves = []
                    for a in range(2):
                        xt = x_pool.tile([128, MT, 1024], FP16,
                                         name=f"x{t}{h}{a}", tag="x")
                        ring.dma_start(
                            xt[:],
                            xd[h].rearrange("(a p) s -> p a s", p=128)
                                 [:, :, bass.ts(a, 1024)])
                        halves.append(xt)
                    st[h][f"x{t}"] = halves
                    wt = wt_pool.tile([128, MT, DH], FP16, name=f"w{t}{h}",
                                      tag=f"w{t}")
                    nc.gpsimd.dma_start(wt[:], wd[h].rearrange("a p d -> p a d"))
                    bt = wt_pool.tile([128, 1], F32, name=f"b{t}{h}", tag=f"b{t}")
                    nc.gpsimd.dma_start(bt[:], bd[h])
                    st[h][f"w{t}"] = wt
                    st[h][f"b{t}"] = bt
                wot = wt_pool.tile([128, DM], FP16, name=f"wo{h}", tag="wo")
                nc.gpsimd.dma_start(wot[:], wo[h])
                st[h]["wo"] = wot

            def emit_proj(h, tensors=("q", "k", "v")):
                """Chunk-pair packed projections; qT/kT duplicated via DMA."""
                names = {"q": "qT", "k": "kT", "v": "vT"}
                for t in tensors:
                    dst = qk_pool.tile([128, S], FP16, name=f"{names[t]}{h}",
                                       tag=names[t])
                    st[h][names[t]] = dst
                    w, b = st[h][f"w{t}"], st[h][f"b{t}"]
                    for pr in range(2):
                        xt = st[h][f"x{t}"][pr]
                        acc = ps_aux.tile([128, 512], F32,
                                          name=f"ac{t}{h}{pr}", tag="a")
                        for mt in range(MT):
                            nc.tensor.matmul(
                                acc[0:DH, :], w[:, mt, :], xt[:, mt, 0:512],
                                start=(mt == 0), stop=(mt == MT - 1),
                                tile_position=(0, 0))
                            nc.tensor.matmul(
                                acc[DH:128, :], w[:, mt, :], xt[:, mt, 512:1024],
                                start=(mt == 0), stop=(mt == MT - 1),
                                tile_position=(0, DH))
                        c0, c1 = 2 * pr, 2 * pr + 1
                        nc.vector.tensor_scalar_add(
                            dst[0:DH, bass.ts(c0, 512)], acc[0:DH, :],
                            b[0:DH])
                        nc.vector.tensor_scalar_add(
                            dst[DH:128, bass.ts(c1, 512)], acc[DH:128, :],
                            b[DH:128])
                        if t != "v":
                            nc.gpsimd.dma_start(dst[DH:128, bass.ts(c0, 512)],
                                                dst[0:DH, bass.ts(c0, 512)])
                            nc.gpsimd.dma_start(dst[0:DH, bass.ts(c1, 512)],
                                                dst[DH:128, bass.ts(c1, 512)])

            def emit_vp(h):
                """PV lhsT [k, d] per k-tile via DMA-transpose of vT."""
                vT = st[h]["vT"]
                vps = []
                for i in range(NT):
                    r0 = 0 if (i // 4) % 2 == 0 else DH
                    vt = vp_pool.tile([128, DH], FP16, name=f"vp{h}_{i}",
                                      tag="vp")
                    nc.sync.dma_start_transpose(
                        vt[:], vT[r0:r0 + DH, bass.ts(i, 128)])
                    vps.append(vt)
                st[h]["vp"] = vps

            def stage_pair(h, i0, qhi):
                """Scores + exp for k-tiles i0, i0+1 (row-packed halves)."""
                qT, kT = st[h]["qT"], st[h]["kT"]
                res = []
                for i, pos in ((i0, 0), (i0 + 1, DH)):
                    qlo = max(128 * i, qhi - 1024)
                    w = qhi - qlo
                    sp = ps_s.tile([128, 1024], F32, name=f"s{h}{i}{qhi}",
                                   tag="s")
                    diag = qlo == 128 * i
                    kTt = kT[pos:pos + DH, bass.ts(i, 128)]
                    for o in range(0, w, 512):
                        ww = min(512, w - o)
                        nc.tensor.matmul(sp[:, o:o + ww], kTt,
                                         qT[pos:pos + DH, qlo + o:qlo + o + ww],
                                         start=True,
                                         stop=not (diag and o == 0))
                        if diag and o == 0:
                            nc.tensor.matmul(sp[:, 0:128], id_sb[:], mask_sb[:],
                                             start=False, stop=True)
                    P = p_pool.tile([128, 1024], FP16, name=f"P{h}{i}{qhi}",
                                    tag="P")
                    nc.scalar.activation(P[:, 0:w], sp[:, 0:w], AF.Exp,
                                         scale=SCALE)
                    res.append((P, qlo))
                return res

            def finish_chunk(h, c, zps):
                zT, rc = st[h]["zT"], st[h]["rc"]
                nc.vector.tensor_copy(zT[0:DH + 1, bass.ts(c, 512)], zps[:])
                rcp = ps_aux.tile([128, 8], FP16, name=f"rcp{h}{c}", tag="a",
                                  padded_shape=[128, 1024])
                for j in range(4):
                    nc.tensor.transpose(
                        rcp[:, 2 * j:2 * j + 1],
                        zT[DH:DH + 1, 512 * c + 128 * j:512 * c + 128 * j + 128],
                        id_sb[DH:DH + 1, DH:DH + 1])
                nc.vector.reciprocal(rc[:, 4 * c:4 * c + 4], rcp[:, 0:8:2])

            def emit_pass(h, cpair, hooks):
                """Attention pass over chunks cpair=(c0,c1); i-major PVs."""
                c0, c1 = cpair
                qhi = 512 * c1 + 512
                nk = 4 * c1 + 4
                vp = st[h]["vp"]
                if c0 == 0:
                    zT = zt_pool.tile([128, S], FP16, name=f"zT{h}", tag="zT")
                    rc = rc_pool.tile([128, NT], F32, name=f"rc{h}", tag="rc")
                    if h < 2:
                        nc.gpsimd.memset(zT[DH:128, :], 0.0)
                    st[h]["zT"] = zT
                    st[h]["rc"] = rc
                z0 = ps_z.tile([DH + 1, 512], F32, name=f"z{h}{c0}", tag="z")
                z1 = ps_z.tile([DH + 1, 512], F32, name=f"z{h}{c1}", tag="z")
                staged = {}
                for i0 in (0, 2):
                    for P, qlo in zip(stage_pair(h, i0, qhi), (i0, i0 + 1)):
                        staged[qlo] = P
                for i in range(nk):
                    if i % 2 == 0 and i + 4 < nk:
                        for P, j in zip(stage_pair(h, i + 4, qhi),
                                        (i + 4, i + 5)):
                            staged[j] = P
                    P, qlo = staged[i]
                    for c, z in ((c0, z0), (c1, z1)):
                        if i >= 4 * c + 4:
                            continue
                        ql = max(512 * c, 128 * i)
                        w = 512 * c + 512 - ql
                        zc = ql - 512 * c
                        Pc = P[:, ql - qlo:ql - qlo + w]
                        nc.tensor.matmul(
                            z[0:DH, zc:zc + w], vp[i][:], Pc,
                            start=(i == 0), stop=(i == 4 * c + 3),
                            tile_position=(0, 0))
                        nc.tensor.matmul(
                            z[DH:DH + 1, zc:zc + w], ones_sb[:], Pc,
                            start=(i == 0), stop=(i == 4 * c + 3),
                            tile_position=(0, DH))
                    del staged[i]
                    if i == 4 * c0 + 3:
                        finish_chunk(h, c0, z0)
                        for f in hooks.get(c0, []):
                            f()
                finish_chunk(h, c1, z1)
                for f in hooks.get(c1, []):
                    f()

            def emit_outproj(h, jjs):
                zT, rc, wot = st[h]["zT"], st[h]["rc"], st[h]["wo"]
                for jj in jjs:
                    ob = ob_pool.tile([128, 2, DM], FP16, name=f"ob{h}{jj}",
                                      tag="ob")
                    for a in range(2):
                        j = 2 * jj + a
                        for mo, mw in ((0, 512), (512, 256)):
                            aps = ps_aux.tile([128, 512], F32,
                                              name=f"o{h}{j}{mo}", tag="a")
                            nc.tensor.matmul(aps[:, 0:mw], zT[:, bass.ts(j, 128)],
                                             wot[:, mo:mo + mw],
                                             start=True, stop=True)
                            nc.vector.tensor_scalar_mul(
                                ob[:, a, mo:mo + mw], aps[:, 0:mw],
                                rc[:, j:j + 1])
                    nc.scalar.dma_start(
                        out[h, bass.ts(jj, 256), :]
                           .rearrange("(a p) m -> p a m", p=128),
                        ob[:])

            emit_loads(0)
            emit_proj(0)
            emit_vp(0)
            for h in range(HPC):
                nxt, prv = h + 1, h - 1
                if nxt < HPC:
                    emit_loads(nxt)
                acts = {0: [], 1: [], 2: [], 3: []}
                if prv >= 0:
                    acts[0].append(lambda p=prv: emit_outproj(p, (0, 1, 2, 3)))
                    acts[1].append(lambda p=prv: emit_outproj(p, (4, 5)))
                    acts[2].append(lambda p=prv: emit_outproj(p, (6, 7)))
                if nxt < HPC:
                    acts[1].append(lambda n=nxt: emit_proj(n, ("q",)))
                    acts[2].append(lambda n=nxt: emit_proj(n, ("k",)))
                    acts[3].append(lambda n=nxt: (emit_proj(n, ("v",)),
                                                  emit_vp(n)))
                if debug and h == 0:
                    nc.gpsimd.dma_start(dqT[:], st[0]["qT"][:])
                    nc.gpsimd.dma_start(dkT[:], st[0]["kT"][:])
                    nc.gpsimd.dma_start(dvT[:], st[0]["vT"][:])
                    for i in range(NT):
                        nc.gpsimd.dma_start(dvp[:, i, :], st[0]["vp"][i][:])
                emit_pass(h, (0, 1), {c: acts[c] for c in (0, 1)})
                emit_pass(h, (2, 3), {c: acts[c] for c in (2, 3)})
                if debug and h == 0:
                    nc.gpsimd.dma_start(dzT[:], st[0]["zT"][:])
                    nc.gpsimd.dma_start(drc[:], st[0]["rc"][:])
            emit_outproj(HPC - 1, tuple(range(8)))
    nc.compile()
    return nc


_CACHED = None


def _program(debug=False):
    global _CACHED
    if _CACHED is None:
        _CACHED = build_program(debug)
    return _CACHED


def _make_in_maps(inputs):
    xq_f = np.asarray(inputs["normalized_resid_pre_q"], dtype=np.float32)
    xk_f = np.asarray(inputs["normalized_resid_pre_k"], dtype=np.float32)
    xv_f = np.asarray(inputs["normalized_resid_pre_v"], dtype=np.float32)
    WQ = np.asarray(inputs["W_Q"], dtype=np.float32)
    WK = np.asarray(inputs["W_K"], dtype=np.float32)
    WV = np.asarray(inputs["W_V"], dtype=np.float32)
    WO = np.asarray(inputs["W_O"], dtype=np.float32)
    bQ = np.asarray(inputs["b_Q"], dtype=np.float32)
    bK = np.asarray(inputs["b_K"], dtype=np.float32)
    bV = np.asarray(inputs["b_V"], dtype=np.float32)
    bO = np.asarray(inputs["b_O"], dtype=np.float32)

    identh = np.eye(128, dtype=np.float16)
    masku = ((np.arange(128)[:, None] > np.arange(128)[None, :])
             .astype(np.float16) * np.float16(NEG))

    def bias2(b):
        # [H, DH] -> [H, 128, 1] duplicated across both partition halves
        out = np.zeros((b.shape[0], 128, 1), np.float32)
        out[:, 0:DH, 0] = b
        out[:, DH:128, 0] = b
        return out

    bQ2, bK2, bV2 = bias2(bQ), bias2(bK), bias2(bV)

    in_maps = []
    for c in range(N_CORES):
        b = c % 2
        hg = c // 2
        hs = slice(HPC * hg, HPC * hg + HPC)
        m = {
            "xq": np.ascontiguousarray(
                xq_f[b, :, hs, :].transpose(1, 2, 0)).astype(np.float16),
            "xk": np.ascontiguousarray(
                xk_f[b, :, hs, :].transpose(1, 2, 0)).astype(np.float16),
            "xv": np.ascontiguousarray(
                xv_f[b, :, hs, :].transpose(1, 2, 0)).astype(np.float16),
            "wq": np.ascontiguousarray(
                WQ[hs].reshape(HPC, MT, 128, DH)).astype(np.float16),
            "wk": np.ascontiguousarray(
                WK[hs].reshape(HPC, MT, 128, DH)).astype(np.float16),
            "wv": np.ascontiguousarray(
                WV[hs].reshape(HPC, MT, 128, DH)).astype(np.float16),
            "wo": np.ascontiguousarray(np.concatenate(
                [WO[hs], np.broadcast_to(bO / H, (HPC, 1, DM)),
                 np.zeros((HPC, 128 - DH - 1, DM), np.float32)],
                axis=1)).astype(np.float16),
            "bq": np.ascontiguousarray(bQ2[hs]),
            "bk": np.ascontiguousarray(bK2[hs]),
            "bv": np.ascontiguousarray(bV2[hs]),
            "identh": identh,
            "masku": masku,
        }
        in_maps.append(m)
    return in_maps


def run(inputs, trace=False, debug=False, **kw):
    nc = _program(debug)
    in_maps = _make_in_maps(inputs)
    res = run_bass_kernel_spmd(nc, in_maps, core_ids=list(range(N_CORES)),
                               trace=trace, **kw)
    full = np.zeros((B, S, H, DM), np.float32)
    for c in range(N_CORES):
        b = c % 2
        hg = c // 2
        o = res.results[c]["out"]
        for j in range(HPC):
            full[b, :, HPC * hg + j, :] = o[j]
    return full, res


def kernel(**inputs):
    full, _ = run(inputs)
    return full


# revision 11
# speedup vs baseline: 1.1154x; 1.1154x over previous
"""Trainium2 Bass kernel for per-head causal attention (nn_Attention_52896817217709).

Sharding: 8 cores = 4 head-groups (3 heads each) x 2 batches.
Per core, per head h (S=2048, D_MODEL=768, D_HEAD=64):
  q&k projected together per 512-chunk, packed on the two PE column halves
  (tile_position (0,0)/(0,64)) -> qkT [128,S] (q rows 0:64, k rows 64:128),
  one full-lane DVE evac per chunk; swap-dup into kqT via SBUF->SBUF DMA so
  the scores matmuls can be 2-way row-packed (K=64 halves, concurrent).
  v self-paired on chunk pairs -> checkerboarded vT; vp (PV lhsT [k,d]) via
  XBAR DMA-transpose into offset-0 pool slots (split sync/scalar rings).
  Causal diag-tile masking is an additive PE matmul (identity^T @ maskU).
  Attention runs in two passes over chunk pairs (q 0:1024 then 1024:2048),
  [128,1024] two-bank PSUM score tiles -> one exp per k-tile (scalar engine
  does only exp).  PV z' [64,512] per chunk plus a concurrent col-packed
  M=1 ones-matmul accumulating softmax sums into z row 64.
  out = (z'^T_j @ [W_O; b_O/H]) * rc_j with rc = 1/sums; evac on DVE; fp16 out.
  xq/xk and W_Q/W_K optionally fp8e4m3 (W scaled x16, absorbed in exp scale).
"""
import sys
import os
import numpy as np

for _p in ("/opt/trn_rl_repo", "/root/.axon_site/_ro/trn_rl_repo"):
    if os.path.isdir(_p) and _p not in sys.path:
        sys.path.insert(0, _p)

import ml_dtypes
import concourse.bass as bass
import concourse.tile as tile
from concourse import bacc, mybir
from concourse.bass_utils import run_bass_kernel_spmd

F32 = mybir.dt.float32
FP16 = mybir.dt.float16
FP8 = mybir.dt.float8e4
AF = mybir.ActivationFunctionType

B, S, H, DM, DH = 2, 2048, 12, 768, 64
HPC = 3            # heads per core
NT = S // 128      # 16 k-tiles
MT = DM // 128     # 6 m-tiles
N_CORES = 8
NEG = -60000.0     # additive causal-mask constant (fp16-safe)

USE_FP8 = True     # xq/xk + W_Q/W_K in fp8e4m3 (x16 weight scale)
WSC = 16.0 if USE_FP8 else 1.0
SCALE = 0.125 / (WSC * WSC)   # exp scale absorbs 1/sqrt(DH) and fp8 scaling
XQK_DT = FP8 if USE_FP8 else FP16
NP_X = ml_dtypes.float8_e4m3fn if USE_FP8 else np.float16


def build_program(debug=False):
    nc = bacc.Bacc("TRN2", target_bir_lowering=False, debug=False)

    xq = nc.dram_tensor("xq", [HPC, DM, S], XQK_DT, kind="ExternalInput")
    xk = nc.dram_tensor("xk", [HPC, DM, S], XQK_DT, kind="ExternalInput")
    xv = nc.dram_tensor("xv", [HPC, DM, S], FP16, kind="ExternalInput")
    wq = nc.dram_tensor("wq", [HPC, MT, 128, DH], XQK_DT, kind="ExternalInput")
    wk = nc.dram_tensor("wk", [HPC, MT, 128, DH], XQK_DT, kind="ExternalInput")
    wv = nc.dram_tensor("wv", [HPC, MT, 128, DH], FP16, kind="ExternalInput")
    wo = nc.dram_tensor("wo", [HPC, 128, DM], FP16, kind="ExternalInput")
    bqk = nc.dram_tensor("bqk", [HPC, 128, 1], F32, kind="ExternalInput")
    bv = nc.dram_tensor("bv", [HPC, 128, 1], F32, kind="ExternalInput")
    identh = nc.dram_tensor("identh", [128, 128], FP16, kind="ExternalInput")
    masku = nc.dram_tensor("masku", [128, 128], FP16, kind="ExternalInput")
    out = nc.dram_tensor("out", [HPC, S, DM], FP16, kind="ExternalOutput")
    if debug:
        dqT = nc.dram_tensor("dqT", [128, S], FP16, kind="ExternalOutput")
        dkT = nc.dram_tensor("dkT", [128, S], FP16, kind="ExternalOutput")
        dvT = nc.dram_tensor("dvT", [128, S], FP16, kind="ExternalOutput")
        dvp = nc.dram_tensor("dvp", [128, NT, DH], FP16, kind="ExternalOutput")
        dzT = nc.dram_tensor("dzT", [128, S], FP16, kind="ExternalOutput")
        drc = nc.dram_tensor("drc", [128, NT], F32, kind="ExternalOutput")

    with tile.TileContext(nc) as tc:
        with (
            tc.tile_pool(name="wpool", bufs=1) as wpool,
            tc.tile_pool(name="xp", bufs=4) as x_pool,
            tc.tile_pool(name="wt", bufs=2) as wt_pool,
            tc.tile_pool(name="qk", bufs=2) as qk_pool,
            tc.tile_pool(name="vp", bufs=24) as vp_pool,
            tc.tile_pool(name="pp", bufs=6) as p_pool,
            tc.tile_pool(name="zt", bufs=2) as zt_pool,
            tc.tile_pool(name="rc", bufs=2) as rc_pool,
            tc.tile_pool(name="ob", bufs=3) as ob_pool,
            tc.tile_pool(name="psa", bufs=2, space="PSUM") as ps_aux,
            tc.tile_pool(name="pss", bufs=2, space="PSUM") as ps_s,
            tc.tile_pool(name="psz", bufs=2, space="PSUM") as ps_z,
        ):
            id_sb = wpool.tile([128, 128], FP16, name="id_sb")
            nc.gpsimd.dma_start(id_sb[:], identh[:])
            mask_sb = wpool.tile([128, 128], FP16, name="mask_sb")
            nc.gpsimd.dma_start(mask_sb[:], masku[:])
            ones_sb = wpool.tile([128, 1], FP16, name="ones_sb")
            nc.gpsimd.memset(ones_sb[:], 1.0)

            st = [dict() for _ in range(HPC)]

            def emit_loads(h):
                """xq on sync; xk/xv/weights on gpsimd; halves for pipelining."""
                for t, xd, ring, dt in (("q", xq, nc.sync, XQK_DT),
                                        ("k", xk, nc.gpsimd, XQK_DT),
                                        ("v", xv, nc.gpsimd, FP16)):
                    halves = []
                    for a in range(2):
                        xt = x_pool.tile([128, MT, 1024], dt,
                                         name=f"x{t}{h}{a}", tag=f"x{t}")
                        ring.dma_start(
                            xt[:],
                            xd[h].rearrange("(a p) s -> p a s", p=128)
                                 [:, :, bass.ts(a, 1024)])
                        halves.append(xt)
                    st[h][f"x{t}"] = halves
                for t, wd in (("q", wq), ("k", wk), ("v", wv)):
                    wt = wt_pool.tile([128, MT, DH],
                                      XQK_DT if t != "v" else FP16,
                                      name=f"w{t}{h}", tag=f"w{t}")
                    nc.gpsimd.dma_start(wt[:], wd[h].rearrange("a p d -> p a d"))
                    st[h][f"w{t}"] = wt
                for t, bd in (("bqk", bqk), ("bv", bv)):
                    bt = wt_pool.tile([128, 1], F32, name=f"{t}{h}", tag=t)
                    nc.gpsimd.dma_start(bt[:], bd[h])
                    st[h][t] = bt
                wot = wt_pool.tile([128, DM], FP16, name=f"wo{h}", tag="wo")
                nc.gpsimd.dma_start(wot[:], wo[h])
                st[h]["wo"] = wot

            def emit_proj_qk(h):
                """q&k col-packed per chunk -> qkT; swap-dup into kqT."""
                qkT = qk_pool.tile([128, S], FP16, name=f"qkT{h}", tag="qkT")
                kqT = qk_pool.tile([128, S], FP16, name=f"kqT{h}", tag="kqT")
                st[h]["qkT"], st[h]["kqT"] = qkT, kqT
                wqt, wkt, b = st[h]["wq"], st[h]["wk"], st[h]["bqk"]
                for c in range(4):
                    xtq = st[h]["xq"][c // 2]
                    xtk = st[h]["xk"][c // 2]
                    off = (c % 2) * 512
                    acc = ps_aux.tile([128, 512], F32, name=f"aqk{h}{c}",
                                      tag="a")
                    for mt in range(MT):
                        nc.tensor.matmul(
                            acc[0:DH, :], wqt[:, mt, :],
                            xtq[:, mt, off:off + 512],
                            start=(mt == 0), stop=(mt == MT - 1),
                            tile_position=(0, 0))
                        nc.tensor.matmul(
                            acc[DH:128, :], wkt[:, mt, :],
                            xtk[:, mt, off:off + 512],
                            start=(mt == 0), stop=(mt == MT - 1),
                            tile_position=(0, DH))
                    nc.vector.tensor_scalar_add(qkT[:, bass.ts(c, 512)],
                                                acc[:], b[:])
                    nc.sync.dma_start(kqT[0:DH, bass.ts(c, 512)],
                                      qkT[DH:128, bass.ts(c, 512)])
                    nc.sync.dma_start(kqT[DH:128, bass.ts(c, 512)],
                                      qkT[0:DH, bass.ts(c, 512)])

            def emit_proj_v(h):
                """v self-paired on chunk pairs -> checkerboarded vT."""
                vT = qk_pool.tile([128, S], FP16, name=f"vT{h}", tag="vT")
                st[h]["vT"] = vT
                w, b = st[h]["wv"], st[h]["bv"]
                for pr in range(2):
                    xt = st[h]["xv"][pr]
                    acc = ps_aux.tile([128, 512], F32, name=f"av{h}{pr}",
                                      tag="a")
                    for mt in range(MT):
                        nc.tensor.matmul(
                            acc[0:DH, :], w[:, mt, :], xt[:, mt, 0:512],
                            start=(mt == 0), stop=(mt == MT - 1),
                            tile_position=(0, 0))
                        nc.tensor.matmul(
                            acc[DH:128, :], w[:, mt, :], xt[:, mt, 512:1024],
                            start=(mt == 0), stop=(mt == MT - 1),
                            tile_position=(0, DH))
                    c0, c1 = 2 * pr, 2 * pr + 1
                    nc.vector.tensor_scalar_add(
                        vT[0:DH, bass.ts(c0, 512)], acc[0:DH, :], b[0:DH])
                    nc.vector.tensor_scalar_add(
                        vT[DH:128, bass.ts(c1, 512)], acc[DH:128, :],
                        b[DH:128])

            def emit_vp(h):
                """PV lhsT [k, d] per k-tile via XBAR transpose (2 rings)."""
                vT = st[h]["vT"]
                vps = []
                for i in range(NT):
                    r0 = 0 if (i // 4) % 2 == 0 else DH
                    vt = vp_pool.tile([128, DH], FP16, name=f"vp{h}_{i}",
                                      tag="vp")
                    ring = nc.sync if i % 2 == 0 else nc.scalar
                    ring.dma_start_transpose(
                        vt[:], vT[r0:r0 + DH, bass.ts(i, 128)])
                    vps.append(vt)
                st[h]["vp"] = vps

            def stage_pair(h, i0, qhi):
                """Scores + exp for k-tiles i0, i0+1 (row-packed halves)."""
                qkT, kqT = st[h]["qkT"], st[h]["kqT"]
                res = []
                for i, pos in ((i0, 0), (i0 + 1, DH)):
                    qlo = max(128 * i, qhi - 1024)
                    w = qhi - qlo
                    sp = ps_s.tile([128, 1024], F32, name=f"s{h}{i}{qhi}",
                                   tag="s")
                    diag = qlo == 128 * i
                    kt = (kqT if pos == 0 else qkT)[pos:pos + DH,
                                                    bass.ts(i, 128)]
                    qt = qkT if pos == 0 else kqT
                    for o in range(0, w, 512):
                        ww = min(512, w - o)
                        nc.tensor.matmul(sp[:, o:o + ww], kt,
                                         qt[pos:pos + DH,
                                            qlo + o:qlo + o + ww],
                                         start=True,
                                         stop=not (diag and o == 0))
                        if diag and o == 0:
                            nc.tensor.matmul(sp[:, 0:128], id_sb[:],
                                             mask_sb[:], start=False,
                                             stop=True)
                    P = p_pool.tile([128, 1024], FP16, name=f"P{h}{i}{qhi}",
                                    tag="P")
                    nc.scalar.activation(P[:, 0:w], sp[:, 0:w], AF.Exp,
                                         scale=SCALE)
                    res.append((P, qlo))
                return res

            def finish_chunk(h, c, zps):
                zT, rc = st[h]["zT"], st[h]["rc"]
                nc.vector.tensor_copy(zT[0:DH + 1, bass.ts(c, 512)], zps[:])
                rcp = ps_aux.tile([128, 8], FP16, name=f"rcp{h}{c}", tag="a",
                                  padded_shape=[128, 1024])
                for j in range(4):
                    nc.tensor.transpose(
                        rcp[:, 2 * j:2 * j + 1],
                        zT[DH:DH + 1,
                           512 * c + 128 * j:512 * c + 128 * j + 128],
                        id_sb[DH:DH + 1, DH:DH + 1])
                nc.vector.reciprocal(rc[:, 4 * c:4 * c + 4], rcp[:, 0:8:2])

            def emit_pass(h, cpair, hooks):
                """Attention pass over chunks cpair=(c0,c1); i-major PVs."""
                c0, c1 = cpair
                qhi = 512 * c1 + 512
                nk = 4 * c1 + 4
                vp = st[h]["vp"]
                if c0 == 0:
                    zT = zt_pool.tile([128, S], FP16, name=f"zT{h}", tag="zT")
                    rc = rc_pool.tile([128, NT], F32, name=f"rc{h}", tag="rc")
                    if h < 2:
                        nc.gpsimd.memset(zT[DH:128, :], 0.0)
                    st[h]["zT"] = zT
                    st[h]["rc"] = rc
                z0 = ps_z.tile([DH + 1, 512], F32, name=f"z{h}{c0}", tag="z")
                z1 = ps_z.tile([DH + 1, 512], F32, name=f"z{h}{c1}", tag="z")
                staged = {}
                for i0 in (0, 2):
                    for P, j in zip(stage_pair(h, i0, qhi), (i0, i0 + 1)):
                        staged[j] = P
                for i in range(nk):
                    if i % 2 == 0 and i + 4 < nk:
                        for P, j in zip(stage_pair(h, i + 4, qhi),
                                        (i + 4, i + 5)):
                            staged[j] = P
                    P, qlo = staged[i]
                    for c, z in ((c0, z0), (c1, z1)):
                        if i >= 4 * c + 4:
                            continue
                        ql = max(512 * c, 128 * i)
                        w = 512 * c + 512 - ql
                        zc = ql - 512 * c
                        Pc = P[:, ql - qlo:ql - qlo + w]
                        nc.tensor.matmul(
                            z[0:DH, zc:zc + w], vp[i][:], Pc,
                            start=(i == 0), stop=(i == 4 * c + 3),
                            tile_position=(0, 0))
                        nc.tensor.matmul(
                            z[DH:DH + 1, zc:zc + w], ones_sb[:], Pc,
                            start=(i == 0), stop=(i == 4 * c + 3),
                            tile_position=(0, DH))
                    del staged[i]
                    if i == 4 * c0 + 3:
                        finish_chunk(h, c0, z0)
                        for f in hooks.get(c0, []):
                            f()
                finish_chunk(h, c1, z1)
                for f in hooks.get(c1, []):
                    f()

            def emit_outproj(h, jjs):
                zT, rc, wot = st[h]["zT"], st[h]["rc"], st[h]["wo"]
                for jj in jjs:
                    ob = ob_pool.tile([128, 2, DM], FP16, name=f"ob{h}{jj}",
                                      tag="ob")
                    for a in range(2):
                        j = 2 * jj + a
                        for mo, mw in ((0, 512), (512, 256)):
                            aps = ps_aux.tile([128, 512], F32,
                                              name=f"o{h}{j}{mo}", tag="a")
                            nc.tensor.matmul(aps[:, 0:mw],
                                             zT[:, bass.ts(j, 128)],
                                             wot[:, mo:mo + mw],
                                             start=True, stop=True)
                            nc.vector.tensor_scalar_mul(
                                ob[:, a, mo:mo + mw], aps[:, 0:mw],
                                rc[:, j:j + 1])
                    nc.scalar.dma_start(
                        out[h, bass.ts(jj, 256), :]
                           .rearrange("(a p) m -> p a m", p=128),
                        ob[:])

            emit_loads(0)
            emit_proj_qk(0)
            emit_proj_v(0)
            emit_vp(0)
            for h in range(HPC):
                nxt, prv = h + 1, h - 1
                if nxt < HPC:
                    emit_loads(nxt)
                acts = {0: [], 1: [], 2: [], 3: []}
                if prv >= 0:
                    acts[0].append(lambda p=prv: emit_outproj(p, (0, 1, 2, 3)))
                    acts[1].append(lambda p=prv: emit_outproj(p, (4, 5)))
                    acts[2].append(lambda p=prv: emit_outproj(p, (6, 7)))
                if nxt < HPC:
                    acts[1].append(lambda n=nxt: emit_proj_qk(n))
                    acts[2].append(lambda n=nxt: emit_proj_v(n))
                    acts[3].append(lambda n=nxt: emit_vp(n))
                if debug and h == 0:
                    nc.gpsimd.dma_start(dqT[:], st[0]["qkT"][:])
                    nc.gpsimd.dma_start(dkT[:], st[0]["kqT"][:])
                    nc.gpsimd.dma_start(dvT[:], st[0]["vT"][:])
                    for i in range(NT):
                        nc.gpsimd.dma_start(dvp[:, i, :], st[0]["vp"][i][:])
                emit_pass(h, (0, 1), {c: acts[c] for c in (0, 1)})
                emit_pass(h, (2, 3), {c: acts[c] for c in (2, 3)})
                if debug and h == 0:
                    nc.gpsimd.dma_start(dzT[:], st[0]["zT"][:])
                    nc.gpsimd.dma_start(drc[:], st[0]["rc"][:])
            emit_outproj(HPC - 1, tuple(range(8)))
    nc.compile()
    return nc


_CACHED = None


def _program(debug=False):
    global _CACHED
    if _CACHED is None:
        _CACHED = build_program(debug)
    return _CACHED


def _make_in_maps(inputs):
    xq_f = np.asarray(inputs["normalized_resid_pre_q"], dtype=np.float32)
    xk_f = np.asarray(inputs["normalized_resid_pre_k"], dtype=np.float32)
    xv_f = np.asarray(inputs["normalized_resid_pre_v"], dtype=np.float32)
    WQ = np.asarray(inputs["W_Q"], dtype=np.float32) * WSC
    WK = np.asarray(inputs["W_K"], dtype=np.float32) * WSC
    WV = np.asarray(inputs["W_V"], dtype=np.float32)
    WO = np.asarray(inputs["W_O"], dtype=np.float32)
    bQ = np.asarray(inputs["b_Q"], dtype=np.float32) * WSC
    bK = np.asarray(inputs["b_K"], dtype=np.float32) * WSC
    bV = np.asarray(inputs["b_V"], dtype=np.float32)
    bO = np.asarray(inputs["b_O"], dtype=np.float32)

    identh = np.eye(128, dtype=np.float16)
    masku = ((np.arange(128)[:, None] > np.arange(128)[None, :])
             .astype(np.float16) * np.float16(NEG))

    bqk2 = np.zeros((H, 128, 1), np.float32)
    bqk2[:, 0:DH, 0] = bQ
    bqk2[:, DH:128, 0] = bK
    bv2 = np.zeros((H, 128, 1), np.float32)
    bv2[:, 0:DH, 0] = bV
    bv2[:, DH:128, 0] = bV

    in_maps = []
    for c in range(N_CORES):
        b = c % 2
        hg = c // 2
        hs = slice(HPC * hg, HPC * hg + HPC)
        m = {
            "xq": np.ascontiguousarray(
                xq_f[b, :, hs, :].transpose(1, 2, 0)).astype(NP_X),
            "xk": np.ascontiguousarray(
                xk_f[b, :, hs, :].transpose(1, 2, 0)).astype(NP_X),
            "xv": np.ascontiguousarray(
                xv_f[b, :, hs, :].transpose(1, 2, 0)).astype(np.float16),
            "wq": np.ascontiguousarray(
                WQ[hs].reshape(HPC, MT, 128, DH)).astype(NP_X),
            "wk": np.ascontiguousarray(
                WK[hs].reshape(HPC, MT, 128, DH)).astype(NP_X),
            "wv": np.ascontiguousarray(
                WV[hs].reshape(HPC, MT, 128, DH)).astype(np.float16),
            "wo": np.ascontiguousarray(np.concatenate(
                [WO[hs], np.broadcast_to(bO / H, (HPC, 1, DM)),
                 np.zeros((HPC, 128 - DH - 1, DM), np.float32)],
                axis=1)).astype(np.float16),
            "bqk": np.ascontiguousarray(bqk2[hs]),
            "bv": np.ascontiguousarray(bv2[hs]),
            "identh": identh,
            "masku": masku,
        }
        in_maps.append(m)
    return in_maps


def run(inputs, trace=False, debug=False, **kw):
    nc = _program(debug)
    in_maps = _make_in_maps(inputs)
    res = run_bass_kernel_spmd(nc, in_maps, core_ids=list(range(N_CORES)),
                               trace=trace, **kw)
    full = np.zeros((B, S, H, DM), np.float32)
    for c in range(N_CORES):
        b = c % 2
        hg = c // 2
        o = res.results[c]["out"]
        for j in range(HPC):
            full[b, :, HPC * hg + j, :] = o[j]
    return full, res


def kernel(**inputs):
    full, _ = run(inputs)
    return full


# revision 14
# speedup vs baseline: 1.1706x; 1.0495x over previous
"""Trainium2 Bass kernel for per-head causal attention (nn_Attention_52896817217709).

Sharding: 8 cores = 4 head-groups (3 heads each) x 2 batches.
Per core, per head h (S=2048, D_MODEL=768, D_HEAD=64):
  q&k projected together per 512-chunk, packed on the two PE column halves
  (tile_position (0,0)/(0,64)) -> qkT [128,S] (q rows 0:64, k rows 64:128),
  one full-lane DVE evac per chunk; swap-dup into kqT via SBUF->SBUF DMA so
  the scores matmuls can be 2-way row-packed (K=64 halves, concurrent).
  v self-paired on chunk pairs -> checkerboarded vT; vp (PV lhsT [k,d]) via
  XBAR DMA-transpose into offset-0 pool slots (split sync/scalar rings).
  Causal diag-tile masking is an additive PE matmul (identity^T @ maskU).
  Attention runs in two passes over chunk pairs (q 0:1024 then 1024:2048),
  [128,1024] two-bank PSUM score tiles -> one exp per k-tile (scalar engine
  does only exp).  PV z' [64,512] per chunk plus a concurrent col-packed
  M=1 ones-matmul accumulating softmax sums into z row 64.
  out = (z'^T_j @ [W_O; b_O/H]) * rc_j with rc = 1/sums; evac on DVE; fp16 out.
  xq/xk and W_Q/W_K optionally fp8e4m3 (W scaled x16, absorbed in exp scale).
"""
import sys
import os
import numpy as np

for _p in ("/opt/trn_rl_repo", "/root/.axon_site/_ro/trn_rl_repo"):
    if os.path.isdir(_p) and _p not in sys.path:
        sys.path.insert(0, _p)

import ml_dtypes
import concourse.bass as bass
import concourse.tile as tile
from concourse import bacc, mybir
from concourse.bass_utils import run_bass_kernel_spmd

F32 = mybir.dt.float32
FP16 = mybir.dt.float16
FP8 = mybir.dt.float8e4
AF = mybir.ActivationFunctionType

B, S, H, DM, DH = 2, 2048, 12, 768, 64
HPC = 3            # heads per core
NT = S // 128      # 16 k-tiles
MT = DM // 128     # 6 m-tiles
N_CORES = 8
NEG = -60000.0     # additive causal-mask constant (fp16-safe)

USE_FP8 = True     # xq/xk + W_Q/W_K in fp8e4m3 (x16 weight scale)
WSC = 16.0 if USE_FP8 else 1.0
SCALE = 0.125 / (WSC * WSC)   # exp scale absorbs 1/sqrt(DH) and fp8 scaling
XQK_DT = FP8 if USE_FP8 else FP16
NP_X = ml_dtypes.float8_e4m3fn if USE_FP8 else np.float16


def build_program(debug=False):
    nc = bacc.Bacc("TRN2", target_bir_lowering=False, debug=False)

    xq = nc.dram_tensor("xq", [HPC, 128, 3, 2, S], XQK_DT,
                        kind="ExternalInput")
    xk = nc.dram_tensor("xk", [HPC, 128, 3, 2, S], XQK_DT,
                        kind="ExternalInput")
    xv = nc.dram_tensor("xv", [HPC, DM, S], FP16, kind="ExternalInput")
    wq = nc.dram_tensor("wq", [HPC, 128, 3, 2, DH], XQK_DT,
                        kind="ExternalInput")
    wk = nc.dram_tensor("wk", [HPC, 128, 3, 2, DH], XQK_DT,
                        kind="ExternalInput")
    wv = nc.dram_tensor("wv", [HPC, MT, 128, DH], FP16, kind="ExternalInput")
    wo = nc.dram_tensor("wo", [HPC, 128, DM], FP16, kind="ExternalInput")
    bq = nc.dram_tensor("bq", [HPC, 128, 1], F32, kind="ExternalInput")
    bk = nc.dram_tensor("bk", [HPC, 128, 1], F32, kind="ExternalInput")
    bv = nc.dram_tensor("bv", [HPC, 128, 1], F32, kind="ExternalInput")
    identh = nc.dram_tensor("identh", [128, 128], FP16, kind="ExternalInput")
    masku = nc.dram_tensor("masku", [128, 128], FP16, kind="ExternalInput")
    out = nc.dram_tensor("out", [HPC, S, DM], FP16, kind="ExternalOutput")
    if debug:
        dqT = nc.dram_tensor("dqT", [128, S], FP16, kind="ExternalOutput")
        dkT = nc.dram_tensor("dkT", [128, S], FP16, kind="ExternalOutput")
        dvT = nc.dram_tensor("dvT", [128, S], FP16, kind="ExternalOutput")
        dvp = nc.dram_tensor("dvp", [128, NT, DH], FP16, kind="ExternalOutput")
        dzT = nc.dram_tensor("dzT", [128, S], FP16, kind="ExternalOutput")
        drc = nc.dram_tensor("drc", [128, NT], F32, kind="ExternalOutput")

    with tile.TileContext(nc) as tc:
        with (
            tc.tile_pool(name="wpool", bufs=1) as wpool,
            tc.tile_pool(name="xp", bufs=4) as x_pool,
            tc.tile_pool(name="wt", bufs=2) as wt_pool,
            tc.tile_pool(name="qk", bufs=2) as qk_pool,
            tc.tile_pool(name="vp", bufs=24) as vp_pool,
            tc.tile_pool(name="pp", bufs=6) as p_pool,
            tc.tile_pool(name="zt", bufs=2) as zt_pool,
            tc.tile_pool(name="rc", bufs=2) as rc_pool,
            tc.tile_pool(name="ob", bufs=3) as ob_pool,
            tc.tile_pool(name="psa", bufs=2, space="PSUM") as ps_aux,
            tc.tile_pool(name="pss", bufs=2, space="PSUM") as ps_s,
            tc.tile_pool(name="psz", bufs=2, space="PSUM") as ps_z,
        ):
            id_sb = wpool.tile([128, 128], FP16, name="id_sb")
            nc.gpsimd.dma_start(id_sb[:], identh[:])
            mask_sb = wpool.tile([128, 128], FP16, name="mask_sb")
            nc.gpsimd.dma_start(mask_sb[:], masku[:])
            ones_sb = wpool.tile([128, 1], FP16, name="ones_sb")
            nc.gpsimd.memset(ones_sb[:], 1.0)

            st = [dict() for _ in range(HPC)]

            def emit_loads(h):
                """xq on sync; xk/xv/weights on gpsimd; halves for pipelining."""
                for t, xd, ring in (("q", xq, nc.sync),
                                    ("k", xk, nc.gpsimd)):
                    halves = []
                    for a in range(2):
                        xt = x_pool.tile([128, 3, 2, 1024], XQK_DT,
                                         name=f"x{t}{h}{a}", tag=f"x{t}")
                        ring.dma_start(
                            xt[:], xd[h][:, :, :, bass.ts(a, 1024)])
                        halves.append(xt)
                    st[h][f"x{t}"] = halves
                halves = []
                for a in range(2):
                    xt = x_pool.tile([128, MT, 1024], FP16,
                                     name=f"xv{h}{a}", tag="xv")
                    nc.gpsimd.dma_start(
                        xt[:],
                        xv[h].rearrange("(a p) s -> p a s", p=128)
                             [:, :, bass.ts(a, 1024)])
                    halves.append(xt)
                st[h]["xv"] = halves
                for t, wd in (("q", wq), ("k", wk)):
                    wt = wt_pool.tile([128, 3, 2, DH], XQK_DT,
                                      name=f"w{t}{h}", tag=f"w{t}")
                    nc.gpsimd.dma_start(wt[:], wd[h][:])
                    st[h][f"w{t}"] = wt
                wt = wt_pool.tile([128, MT, DH], FP16, name=f"wv{h}", tag="wv")
                nc.gpsimd.dma_start(wt[:], wv[h].rearrange("a p d -> p a d"))
                st[h]["wv"] = wt
                for t, bd in (("bq", bq), ("bk", bk), ("bv", bv)):
                    bt = wt_pool.tile([128, 1], F32, name=f"{t}{h}", tag=t)
                    nc.gpsimd.dma_start(bt[:], bd[h])
                    st[h][t] = bt
                wot = wt_pool.tile([128, DM], FP16, name=f"wo{h}", tag="wo")
                nc.gpsimd.dma_start(wot[:], wo[h])
                st[h]["wo"] = wot

            def emit_proj_qk(h):
                """q,k DoubleRow projections -> qT/kT, dup'd to both halves."""
                qT = qk_pool.tile([128, S], FP16, name=f"qT{h}", tag="qT")
                kT = qk_pool.tile([128, S], FP16, name=f"kT{h}", tag="kT")
                st[h]["qT"], st[h]["kT"] = qT, kT
                DR = mybir.MatmulPerfMode.DoubleRow
                for c in range(4):
                    off = (c % 2) * 512
                    for t, dst in (("q", qT), ("k", kT)):
                        xt = st[h][f"x{t}"][c // 2]
                        wt = st[h][f"w{t}"]
                        b = st[h][f"b{t}"]
                        acc = ps_aux.tile([128, 512], F32,
                                          name=f"a{t}{h}{c}", tag="a")
                        for bb in range(3):
                            nc.tensor.matmul(
                                acc[0:DH, :], wt[:, bb, :, :],
                                xt[:, bb, :, off:off + 512],
                                start=(bb == 0), stop=(bb == 2),
                                perf_mode=DR)
                        nc.vector.tensor_scalar_add(
                            dst[0:DH, bass.ts(c, 512)], acc[0:DH, :],
                            b[0:DH])
                        nc.sync.dma_start(dst[DH:128, bass.ts(c, 512)],
                                          dst[0:DH, bass.ts(c, 512)])

            def emit_proj_v(h):
                """v self-paired on chunk pairs -> checkerboarded vT."""
                vT = qk_pool.tile([128, S], FP16, name=f"vT{h}", tag="vT")
                st[h]["vT"] = vT
                w, b = st[h]["wv"], st[h]["bv"]
                for pr in range(2):
                    xt = st[h]["xv"][pr]
                    acc = ps_aux.tile([128, 512], F32, name=f"av{h}{pr}",
                                      tag="a")
                    for mt in range(MT):
                        nc.tensor.matmul(
                            acc[0:DH, :], w[:, mt, :], xt[:, mt, 0:512],
                            start=(mt == 0), stop=(mt == MT - 1),
                            tile_position=(0, 0))
                        nc.tensor.matmul(
                            acc[DH:128, :], w[:, mt, :], xt[:, mt, 512:1024],
                            start=(mt == 0), stop=(mt == MT - 1),
                            tile_position=(0, DH))
                    c0, c1 = 2 * pr, 2 * pr + 1
                    nc.vector.tensor_scalar_add(
                        vT[0:DH, bass.ts(c0, 512)], acc[0:DH, :], b[0:DH])
                    nc.vector.tensor_scalar_add(
                        vT[DH:128, bass.ts(c1, 512)], acc[DH:128, :],
                        b[DH:128])

            def emit_vp(h):
                """PV lhsT [k, d] per k-tile via XBAR transpose (2 rings)."""
                vT = st[h]["vT"]
                vps = []
                for i in range(NT):
                    r0 = 0 if (i // 4) % 2 == 0 else DH
                    vt = vp_pool.tile([128, DH + 4], FP16, name=f"vp{h}_{i}",
                                      tag="vp")
                    ring = nc.sync if i % 2 == 0 else nc.scalar
                    ring.dma_start_transpose(
                        vt[:, 0:DH], vT[r0:r0 + DH, bass.ts(i, 128)])
                    nc.gpsimd.memset(vt[:, DH:DH + 1], 1.0)
                    vps.append(vt)
                st[h]["vp"] = vps

            def stage_pair(h, i0, qhi):
                """Scores + exp for k-tiles i0, i0+1 (row-packed halves)."""
                qT, kT = st[h]["qT"], st[h]["kT"]
                res = []
                for i, pos in ((i0, 0), (i0 + 1, DH)):
                    qlo = max(128 * i, qhi - 1024)
                    w = qhi - qlo
                    sp = ps_s.tile([128, 1024], F32, name=f"s{h}{i}{qhi}",
                                   tag="s")
                    diag = qlo == 128 * i
                    kt = kT[pos:pos + DH, bass.ts(i, 128)]
                    qt = qT
                    for o in range(0, w, 512):
                        ww = min(512, w - o)
                        nc.tensor.matmul(sp[:, o:o + ww], kt,
                                         qt[pos:pos + DH,
                                            qlo + o:qlo + o + ww],
                                         start=True,
                                         stop=not (diag and o == 0))
                        if diag and o == 0:
                            nc.tensor.matmul(sp[:, 0:128], id_sb[:],
                                             mask_sb[:], start=False,
                                             stop=True)
                    P = p_pool.tile([128, 1024], FP16, name=f"P{h}{i}{qhi}",
                                    tag="P")
                    nc.scalar.activation(P[:, 0:w], sp[:, 0:w], AF.Exp,
                                         scale=SCALE)
                    res.append((P, qlo))
                return res

            def finish_chunk(h, c, zps):
                zT, rc = st[h]["zT"], st[h]["rc"]
                nc.vector.tensor_copy(zT[0:DH + 1, bass.ts(c, 512)], zps[:])
                rcp = ps_aux.tile([128, 8], FP16, name=f"rcp{h}{c}", tag="a",
                                  padded_shape=[128, 1024])
                for j in range(4):
                    nc.tensor.transpose(
                        rcp[:, 2 * j:2 * j + 1],
                        zT[DH:DH + 1,
                           512 * c + 128 * j:512 * c + 128 * j + 128],
                        id_sb[DH:DH + 1, DH:DH + 1])
                nc.vector.reciprocal(rc[:, 4 * c:4 * c + 4], rcp[:, 0:8:2])
                nc.sync.dma_start(zT[DH:128, bass.ts(c, 512)],
                                  zT[0:DH, bass.ts(c, 512)])

            def emit_pass(h, cpair, hooks):
                """Attention pass over chunks cpair=(c0,c1); i-major PVs."""
                c0, c1 = cpair
                qhi = 512 * c1 + 512
                nk = 4 * c1 + 4
                vp = st[h]["vp"]
                if c0 == 0:
                    zT = zt_pool.tile([128, S], FP16, name=f"zT{h}", tag="zT")
                    rc = rc_pool.tile([128, NT], F32, name=f"rc{h}", tag="rc")
                    st[h]["zT"] = zT
                    st[h]["rc"] = rc
                z0 = ps_z.tile([DH + 1, 512], F32, name=f"z{h}{c0}", tag="z")
                z1 = ps_z.tile([DH + 1, 512], F32, name=f"z{h}{c1}", tag="z")
                staged = {}
                for i0 in (0, 2):
                    for P, j in zip(stage_pair(h, i0, qhi), (i0, i0 + 1)):
                        staged[j] = P
                for i in range(nk):
                    if i % 2 == 0 and i + 4 < nk:
                        for P, j in zip(stage_pair(h, i + 4, qhi),
                                        (i + 4, i + 5)):
                            staged[j] = P
                    P, qlo = staged[i]
                    for c, z in ((c0, z0), (c1, z1)):
                        if i >= 4 * c + 4:
                            continue
                        ql = max(512 * c, 128 * i)
                        w = 512 * c + 512 - ql
                        zc = ql - 512 * c
                        Pc = P[:, ql - qlo:ql - qlo + w]
                        nc.tensor.matmul(
                            z[:, zc:zc + w], vp[i][:, 0:DH + 1], Pc,
                            start=(i == 0), stop=(i == 4 * c + 3))
                    del staged[i]
                    if i == 4 * c0 + 3:
                        finish_chunk(h, c0, z0)
                        for f in hooks.get(c0, []):
                            f()
                finish_chunk(h, c1, z1)
                for f in hooks.get(c1, []):
                    f()

            def emit_outproj(h, jjs):
                zT, rc, wot = st[h]["zT"], st[h]["rc"], st[h]["wo"]
                for jj in jjs:
                    ob = ob_pool.tile([128, 2, DM], FP16, name=f"ob{h}{jj}",
                                      tag="ob")
                    j0, j1 = 2 * jj, 2 * jj + 1
                    for mo, mw in ((0, 512), (512, 256)):
                        apsA = ps_aux.tile([128, 512], F32,
                                           name=f"o{h}{j0}{mo}", tag="a")
                        apsB = ps_aux.tile([128, 512], F32,
                                           name=f"o{h}{j1}{mo}", tag="a")
                        nc.tensor.matmul(apsA[:, 0:mw],
                                         zT[0:DH, bass.ts(j0, 128)],
                                         wot[0:DH, mo:mo + mw],
                                         start=True, stop=True)
                        nc.tensor.matmul(apsB[:, 0:mw],
                                         zT[DH:128, bass.ts(j1, 128)],
                                         wot[DH:128, mo:mo + mw],
                                         start=True, stop=True)
                        nc.vector.tensor_scalar_mul(
                            ob[:, 0, mo:mo + mw], apsA[:, 0:mw],
                            rc[:, j0:j0 + 1])
                        nc.vector.tensor_scalar_mul(
                            ob[:, 1, mo:mo + mw], apsB[:, 0:mw],
                            rc[:, j1:j1 + 1])
                    nc.scalar.dma_start(
                        out[h, bass.ts(jj, 256), :]
                           .rearrange("(a p) m -> p a m", p=128),
                        ob[:])

            emit_loads(0)
            emit_proj_qk(0)
            emit_proj_v(0)
            emit_vp(0)
            for h in range(HPC):
                nxt, prv = h + 1, h - 1
                if nxt < HPC:
                    emit_loads(nxt)
                acts = {0: [], 1: [], 2: [], 3: []}
                if prv >= 0:
                    acts[0].append(lambda p=prv: emit_outproj(p, (0, 1, 2, 3)))
                    acts[1].append(lambda p=prv: emit_outproj(p, (4, 5)))
                    acts[2].append(lambda p=prv: emit_outproj(p, (6, 7)))
                if nxt < HPC:
                    acts[1].append(lambda n=nxt: emit_proj_qk(n))
                    acts[2].append(lambda n=nxt: emit_proj_v(n))
                    acts[3].append(lambda n=nxt: emit_vp(n))
                if debug and h == 0:
                    nc.gpsimd.dma_start(dqT[:], st[0]["qT"][:])
                    nc.gpsimd.dma_start(dkT[:], st[0]["kT"][:])
                    nc.gpsimd.dma_start(dvT[:], st[0]["vT"][:])
                    for i in range(NT):
                        nc.gpsimd.dma_start(dvp[:, i, :], st[0]["vp"][i][:])
                emit_pass(h, (0, 1), {c: acts[c] for c in (0, 1)})
                emit_pass(h, (2, 3), {c: acts[c] for c in (2, 3)})
                if debug and h == 0:
                    nc.gpsimd.dma_start(dzT[:], st[0]["zT"][:])
                    nc.gpsimd.dma_start(drc[:], st[0]["rc"][:])
            emit_outproj(HPC - 1, tuple(range(8)))
    nc.compile()
    return nc


_CACHED = None


def _program(debug=False):
    global _CACHED
    if _CACHED is None:
        _CACHED = build_program(debug)
    return _CACHED


def _make_in_maps(inputs):
    xq_f = np.asarray(inputs["normalized_resid_pre_q"], dtype=np.float32)
    xk_f = np.asarray(inputs["normalized_resid_pre_k"], dtype=np.float32)
    xv_f = np.asarray(inputs["normalized_resid_pre_v"], dtype=np.float32)
    WQ = np.asarray(inputs["W_Q"], dtype=np.float32) * WSC
    WK = np.asarray(inputs["W_K"], dtype=np.float32) * WSC
    WV = np.asarray(inputs["W_V"], dtype=np.float32)
    WO = np.asarray(inputs["W_O"], dtype=np.float32)
    bQ = np.asarray(inputs["b_Q"], dtype=np.float32) * WSC
    bK = np.asarray(inputs["b_K"], dtype=np.float32) * WSC
    bV = np.asarray(inputs["b_V"], dtype=np.float32)
    bO = np.asarray(inputs["b_O"], dtype=np.float32)

    def interleave_x(x):  # [DM, S] -> [128, 3, 2, S], m = 256b + 128k + p
        return np.ascontiguousarray(
            x.reshape(3, 2, 128, -1).transpose(2, 0, 1, 3))

    def interleave_w(w):  # [DM, DH] -> [128, 3, 2, DH]
        return np.ascontiguousarray(
            w.reshape(3, 2, 128, DH).transpose(2, 0, 1, 3))

    identh = np.eye(128, dtype=np.float16)
    masku = ((np.arange(128)[:, None] > np.arange(128)[None, :])
             .astype(np.float16) * np.float16(NEG))

    bq2 = np.zeros((H, 128, 1), np.float32)
    bq2[:, 0:DH, 0] = bQ
    bq2[:, DH:128, 0] = bQ
    bk2 = np.zeros((H, 128, 1), np.float32)
    bk2[:, 0:DH, 0] = bK
    bk2[:, DH:128, 0] = bK
    bv2 = np.zeros((H, 128, 1), np.float32)
    bv2[:, 0:DH, 0] = bV
    bv2[:, DH:128, 0] = bV

    in_maps = []
    for c in range(N_CORES):
        b = c % 2
        hg = c // 2
        hs = slice(HPC * hg, HPC * hg + HPC)
        m = {
            "xq": np.stack([interleave_x(
                xq_f[b, :, HPC * hg + j, :].T) for j in range(HPC)]
                ).astype(NP_X),
            "xk": np.stack([interleave_x(
                xk_f[b, :, HPC * hg + j, :].T) for j in range(HPC)]
                ).astype(NP_X),
            "xv": np.ascontiguousarray(
                xv_f[b, :, hs, :].transpose(1, 2, 0)).astype(np.float16),
            "wq": np.stack([interleave_w(WQ[HPC * hg + j])
                            for j in range(HPC)]).astype(NP_X),
            "wk": np.stack([interleave_w(WK[HPC * hg + j])
                            for j in range(HPC)]).astype(NP_X),
            "wv": np.ascontiguousarray(
                WV[hs].reshape(HPC, MT, 128, DH)).astype(np.float16),
            "wo": np.ascontiguousarray(np.concatenate(
                [WO[hs], WO[hs]], axis=1)).astype(np.float16),
            "bq": np.ascontiguousarray(bq2[hs]),
            "bk": np.ascontiguousarray(bk2[hs]),
            "bv": np.ascontiguousarray(bv2[hs]),
            "identh": identh,
            "masku": masku,
        }
        in_maps.append(m)
    return in_maps


def run(inputs, trace=False, debug=False, **kw):
    nc = _program(debug)
    in_maps = _make_in_maps(inputs)
    res = run_bass_kernel_spmd(nc, in_maps, core_ids=list(range(N_CORES)),
                               trace=trace, **kw)
    full = np.zeros((B, S, H, DM), np.float32)
    for c in range(N_CORES):
        b = c % 2
        hg = c // 2
        o = res.results[c]["out"]
        for j in range(HPC):
            full[b, :, HPC * hg + j, :] = o[j]
    bO = np.asarray(inputs["b_O"], dtype=np.float32)
    if np.any(bO):
        full += bO / H
    return full, res


def kernel(**inputs):
    full, _ = run(inputs)
    return full


# revision 18
# speedup vs baseline: 1.4727x; 1.2581x over previous
"""Trainium2 Bass kernel for per-head causal attention (nn_Attention_52896817217709).

Sharding: 8 cores = 4 head-groups (3 heads each) x 2 batches.
Per core, per head h (S=2048, D_MODEL=768, D_HEAD=64):
  q&k projected together per 512-chunk, packed on the two PE column halves
  (tile_position (0,0)/(0,64)) -> qkT [128,S] (q rows 0:64, k rows 64:128),
  one full-lane DVE evac per chunk; swap-dup into kqT via SBUF->SBUF DMA so
  the scores matmuls can be 2-way row-packed (K=64 halves, concurrent).
  v self-paired on chunk pairs -> checkerboarded vT; vp (PV lhsT [k,d]) via
  XBAR DMA-transpose into offset-0 pool slots (split sync/scalar rings).
  Causal diag-tile masking is an additive PE matmul (identity^T @ maskU).
  Attention runs in two passes over chunk pairs (q 0:1024 then 1024:2048),
  [128,1024] two-bank PSUM score tiles -> one exp per k-tile (scalar engine
  does only exp).  PV z' [64,512] per chunk plus a concurrent col-packed
  M=1 ones-matmul accumulating softmax sums into z row 64.
  out = (z'^T_j @ [W_O; b_O/H]) * rc_j with rc = 1/sums; evac on DVE; fp16 out.
  xq/xk and W_Q/W_K optionally fp8e4m3 (W scaled x16, absorbed in exp scale).
"""
import sys
import os
import numpy as np

for _p in ("/opt/trn_rl_repo", "/root/.axon_site/_ro/trn_rl_repo"):
    if os.path.isdir(_p) and _p not in sys.path:
        sys.path.insert(0, _p)

import ml_dtypes
import concourse.bass as bass
import concourse.tile as tile
from concourse import bacc, mybir
from concourse.bass_utils import run_bass_kernel_spmd

F32 = mybir.dt.float32
FP16 = mybir.dt.float16
FP8 = mybir.dt.float8e4
AF = mybir.ActivationFunctionType

B, S, H, DM, DH = 2, 2048, 12, 768, 64
HPC = 3            # heads per core
NT = S // 128      # 16 k-tiles
MT = DM // 128     # 6 m-tiles
N_CORES = 8
NEG = -60000.0     # additive causal-mask constant (fp16-safe)

USE_FP8 = True     # xq/xk + W_Q/W_K in fp8e4m3 (x16 weight scale)
WSC = 16.0 if USE_FP8 else 1.0
SCALE = 0.125 / (WSC * WSC)   # exp scale absorbs 1/sqrt(DH) and fp8 scaling
XQK_DT = FP8 if USE_FP8 else FP16
NP_X = ml_dtypes.float8_e4m3fn if USE_FP8 else np.float16


def build_program(debug=False):
    nc = bacc.Bacc("TRN2", target_bir_lowering=False, debug=False)

    xq = nc.dram_tensor("xq", [HPC, 128, 3, 2, S], XQK_DT,
                        kind="ExternalInput")
    xk = nc.dram_tensor("xk", [HPC, 128, 3, 2, S], XQK_DT,
                        kind="ExternalInput")
    xv = nc.dram_tensor("xv", [HPC, DM, S], FP16, kind="ExternalInput")
    wq = nc.dram_tensor("wq", [HPC, 128, 3, 2, DH], XQK_DT,
                        kind="ExternalInput")
    wk = nc.dram_tensor("wk", [HPC, 128, 3, 2, DH], XQK_DT,
                        kind="ExternalInput")
    wv = nc.dram_tensor("wv", [HPC, MT, 128, DH], FP16, kind="ExternalInput")
    wo = nc.dram_tensor("wo", [HPC, 128, DM], FP16, kind="ExternalInput")
    bq = nc.dram_tensor("bq", [HPC, 128, 1], F32, kind="ExternalInput")
    bk = nc.dram_tensor("bk", [HPC, 128, 1], F32, kind="ExternalInput")
    bv = nc.dram_tensor("bv", [HPC, 128, 1], F32, kind="ExternalInput")
    identh = nc.dram_tensor("identh", [128, 128], FP16, kind="ExternalInput")
    masku = nc.dram_tensor("masku", [128, 128], FP16, kind="ExternalInput")
    out = nc.dram_tensor("out", [HPC, S, DM], FP16, kind="ExternalOutput")
    if debug:
        dqT = nc.dram_tensor("dqT", [128, S], FP16, kind="ExternalOutput")
        dkT = nc.dram_tensor("dkT", [128, S], FP16, kind="ExternalOutput")
        dvT = nc.dram_tensor("dvT", [128, S], FP16, kind="ExternalOutput")
        dvp = nc.dram_tensor("dvp", [128, NT, DH], FP16, kind="ExternalOutput")
        dzT = nc.dram_tensor("dzT", [128, S], FP16, kind="ExternalOutput")
        drc = nc.dram_tensor("drc", [128, NT], F32, kind="ExternalOutput")

    with tile.TileContext(nc) as tc:
        with (
            tc.tile_pool(name="wpool", bufs=1) as wpool,
            tc.tile_pool(name="xp", bufs=4) as x_pool,
            tc.tile_pool(name="wt", bufs=2) as wt_pool,
            tc.tile_pool(name="qk", bufs=2) as qk_pool,
            tc.tile_pool(name="vp", bufs=24) as vp_pool,
            tc.tile_pool(name="pp", bufs=6) as p_pool,
            tc.tile_pool(name="zt", bufs=2) as zt_pool,
            tc.tile_pool(name="rc", bufs=2) as rc_pool,
            tc.tile_pool(name="ob", bufs=3) as ob_pool,
            tc.tile_pool(name="psa", bufs=2, space="PSUM") as ps_aux,
            tc.tile_pool(name="pss", bufs=2, space="PSUM") as ps_s,
            tc.tile_pool(name="psz", bufs=2, space="PSUM") as ps_z,
        ):
            id_sb = wpool.tile([128, 128], FP16, name="id_sb")
            nc.gpsimd.dma_start(id_sb[:], identh[:])
            mask_sb = wpool.tile([128, 128], FP16, name="mask_sb")
            nc.gpsimd.dma_start(mask_sb[:], masku[:])
            ones_sb = wpool.tile([128, 1], FP16, name="ones_sb")
            nc.gpsimd.memset(ones_sb[:], 1.0)

            st = [dict() for _ in range(HPC)]

            def emit_loads(h):
                """Weights first (sync); xq on sync, xk/xv on gpsimd."""
                for t, wd in (("q", wq), ("k", wk)):
                    wt = wt_pool.tile([128, 3, 2, DH], XQK_DT,
                                      name=f"w{t}{h}", tag=f"w{t}")
                    nc.sync.dma_start(wt[:], wd[h][:])
                    st[h][f"w{t}"] = wt
                wt = wt_pool.tile([128, MT, DH], FP16, name=f"wv{h}", tag="wv")
                nc.sync.dma_start(wt[:], wv[h].rearrange("a p d -> p a d"))
                st[h]["wv"] = wt
                for t, bd in (("bq", bq), ("bk", bk), ("bv", bv)):
                    bt = wt_pool.tile([128, 1], F32, name=f"{t}{h}", tag=t)
                    nc.sync.dma_start(bt[:], bd[h])
                    st[h][t] = bt
                wot = wt_pool.tile([128, DM], FP16, name=f"wo{h}", tag="wo")
                nc.sync.dma_start(wot[:], wo[h])
                st[h]["wo"] = wot
                for t, xd, ring in (("q", xq, nc.sync),
                                    ("k", xk, nc.gpsimd)):
                    halves = []
                    for a in range(2):
                        xt = x_pool.tile([128, 3, 2, 1024], XQK_DT,
                                         name=f"x{t}{h}{a}", tag=f"x{t}")
                        ring.dma_start(
                            xt[:], xd[h][:, :, :, bass.ts(a, 1024)])
                        halves.append(xt)
                    st[h][f"x{t}"] = halves
                halves = []
                for a in range(2):
                    xt = x_pool.tile([128, MT, 1024], FP16,
                                     name=f"xv{h}{a}", tag="xv")
                    nc.gpsimd.dma_start(
                        xt[:],
                        xv[h].rearrange("(a p) s -> p a s", p=128)
                             [:, :, bass.ts(a, 1024)])
                    halves.append(xt)
                st[h]["xv"] = halves

            def emit_proj_qk(h):
                """q,k DoubleRow projections -> qT/kT, dup'd to both halves."""
                qT = qk_pool.tile([128, S], FP16, name=f"qT{h}", tag="qT")
                kT = qk_pool.tile([128, S], FP16, name=f"kT{h}", tag="kT")
                st[h]["qT"], st[h]["kT"] = qT, kT
                DR = mybir.MatmulPerfMode.DoubleRow
                for c in range(4):
                    off = (c % 2) * 512
                    for t, dst in (("q", qT), ("k", kT)):
                        xt = st[h][f"x{t}"][c // 2]
                        wt = st[h][f"w{t}"]
                        b = st[h][f"b{t}"]
                        acc = ps_aux.tile([128, 512], F32,
                                          name=f"a{t}{h}{c}", tag="a")
                        for bb in range(3):
                            nc.tensor.matmul(
                                acc[0:DH, :], wt[:, bb, :, :],
                                xt[:, bb, :, off:off + 512],
                                start=(bb == 0), stop=(bb == 2),
                                perf_mode=DR)
                        nc.vector.tensor_scalar_add(
                            dst[0:DH, bass.ts(c, 512)], acc[0:DH, :],
                            b[0:DH])
                        nc.sync.dma_start(dst[DH:128, bass.ts(c, 512)],
                                          dst[0:DH, bass.ts(c, 512)])

            def emit_proj_v(h):
                """v self-paired on chunk pairs -> checkerboarded vT."""
                vT = qk_pool.tile([128, S], FP16, name=f"vT{h}", tag="vT")
                st[h]["vT"] = vT
                w, b = st[h]["wv"], st[h]["bv"]
                for pr in range(2):
                    xt = st[h]["xv"][pr]
                    acc = ps_aux.tile([128, 512], F32, name=f"av{h}{pr}",
                                      tag="a")
                    for mt in range(MT):
                        nc.tensor.matmul(
                            acc[0:DH, :], w[:, mt, :], xt[:, mt, 0:512],
                            start=(mt == 0), stop=(mt == MT - 1),
                            tile_position=(0, 0))
                        nc.tensor.matmul(
                            acc[DH:128, :], w[:, mt, :], xt[:, mt, 512:1024],
                            start=(mt == 0), stop=(mt == MT - 1),
                            tile_position=(0, DH))
                    c0, c1 = 2 * pr, 2 * pr + 1
                    nc.vector.tensor_scalar_add(
                        vT[0:DH, bass.ts(c0, 512)], acc[0:DH, :], b[0:DH])
                    nc.vector.tensor_scalar_add(
                        vT[DH:128, bass.ts(c1, 512)], acc[DH:128, :],
                        b[DH:128])

            def emit_vp(h):
                """PV lhsT [k, d|1] per k-tile via PE transpose + DVE copy."""
                vT = st[h]["vT"]
                vps = []
                for i in range(NT):
                    r0 = 0 if (i // 4) % 2 == 0 else DH
                    vt = vp_pool.tile([128, DH + 4], FP16, name=f"vp{h}_{i}",
                                      tag="vp")
                    v_ps = ps_aux.tile([128, DH], FP16, name=f"vps{h}{i}",
                                       tag="a", padded_shape=[128, 1024])
                    nc.tensor.transpose(v_ps[:], vT[r0:r0 + DH, bass.ts(i, 128)],
                                        id_sb[r0:r0 + DH, r0:r0 + DH])
                    nc.vector.tensor_copy(vt[:, 0:DH], v_ps[:])
                    nc.gpsimd.memset(vt[:, DH:DH + 1], 1.0)
                    vps.append(vt)
                st[h]["vp"] = vps

            def stage_pair(h, i0, qhi):
                """Scores + exp for k-tiles i0, i0+1 (row-packed halves)."""
                qT, kT = st[h]["qT"], st[h]["kT"]
                res = []
                for i, pos in ((i0, 0), (i0 + 1, DH)):
                    qlo = max(128 * i, qhi - 1024)
                    w = qhi - qlo
                    sp = ps_s.tile([128, 1024], F32, name=f"s{h}{i}{qhi}",
                                   tag="s")
                    diag = qlo == 128 * i
                    kt = kT[pos:pos + DH, bass.ts(i, 128)]
                    qt = qT
                    for o in range(0, w, 512):
                        ww = min(512, w - o)
                        nc.tensor.matmul(sp[:, o:o + ww], kt,
                                         qt[pos:pos + DH,
                                            qlo + o:qlo + o + ww],
                                         start=True,
                                         stop=not (diag and o == 0))
                        if diag and o == 0:
                            nc.tensor.matmul(sp[:, 0:128], id_sb[:],
                                             mask_sb[:], start=False,
                                             stop=True)
                    P = p_pool.tile([128, 1024], FP16, name=f"P{h}{i}{qhi}",
                                    tag="P")
                    nc.scalar.activation(P[:, 0:w], sp[:, 0:w], AF.Exp,
                                         scale=SCALE)
                    res.append((P, qlo))
                return res

            def finish_chunk(h, c, zps):
                zT, rc, srow = st[h]["zT"], st[h]["rc"], st[h]["srow"]
                nc.vector.tensor_copy(zT[0:DH, bass.ts(c, 512)], zps[0:DH, :])
                nc.vector.tensor_copy(srow[DH:DH + 1, bass.ts(c, 512)],
                                      zps[DH:DH + 1, :])
                rcp = ps_aux.tile([128, 8], FP16, name=f"rcp{h}{c}", tag="a",
                                  padded_shape=[128, 1024])
                for j in range(4):
                    nc.tensor.transpose(
                        rcp[:, 2 * j:2 * j + 1],
                        srow[DH:DH + 1,
                             512 * c + 128 * j:512 * c + 128 * j + 128],
                        id_sb[DH:DH + 1, DH:DH + 1])
                nc.vector.reciprocal(rc[:, 4 * c:4 * c + 4], rcp[:, 0:8:2])
                nc.sync.dma_start(zT[DH:128, bass.ts(c, 512)],
                                  zT[0:DH, bass.ts(c, 512)])

            def emit_pass(h, cpair, hooks):
                """Attention pass over chunks cpair=(c0,c1); i-major PVs."""
                c0, c1 = cpair
                qhi = 512 * c1 + 512
                nk = 4 * c1 + 4
                vp = st[h]["vp"]
                if c0 == 0:
                    zT = zt_pool.tile([128, S], FP16, name=f"zT{h}", tag="zT")
                    rc = rc_pool.tile([128, NT], F32, name=f"rc{h}", tag="rc")
                    srow = rc_pool.tile([DH + 1, S], FP16, name=f"srow{h}",
                                        tag="srow")
                    st[h]["srow"] = srow
                    st[h]["zT"] = zT
                    st[h]["rc"] = rc
                z0 = ps_z.tile([DH + 1, 512], F32, name=f"z{h}{c0}", tag="z")
                z1 = ps_z.tile([DH + 1, 512], F32, name=f"z{h}{c1}", tag="z")
                staged = {}
                for i0 in (0, 2):
                    for P, j in zip(stage_pair(h, i0, qhi), (i0, i0 + 1)):
                        staged[j] = P
                for i in range(nk):
                    if i % 2 == 0 and i + 4 < nk:
                        for P, j in zip(stage_pair(h, i + 4, qhi),
                                        (i + 4, i + 5)):
                            staged[j] = P
                    P, qlo = staged[i]
                    for c, z in ((c0, z0), (c1, z1)):
                        if i >= 4 * c + 4:
                            continue
                        ql = max(512 * c, 128 * i)
                        w = 512 * c + 512 - ql
                        zc = ql - 512 * c
                        Pc = P[:, ql - qlo:ql - qlo + w]
                        nc.tensor.matmul(
                            z[:, zc:zc + w], vp[i][:, 0:DH + 1], Pc,
                            start=(i == 0), stop=(i == 4 * c + 3))
                    del staged[i]
                    if i == 4 * c0 + 3:
                        finish_chunk(h, c0, z0)
                        for f in hooks.get(c0, []):
                            f()
                finish_chunk(h, c1, z1)
                for f in hooks.get(c1, []):
                    f()

            def emit_outproj(h, jjs):
                zT, rc, wot = st[h]["zT"], st[h]["rc"], st[h]["wo"]
                for jj in jjs:
                    ob = ob_pool.tile([128, 2, DM], FP16, name=f"ob{h}{jj}",
                                      tag="ob")
                    j0, j1 = 2 * jj, 2 * jj + 1
                    for mo, mw in ((0, 512), (512, 256)):
                        apsA = ps_aux.tile([128, 512], F32,
                                           name=f"o{h}{j0}{mo}", tag="a")
                        apsB = ps_aux.tile([128, 512], F32,
                                           name=f"o{h}{j1}{mo}", tag="a")
                        nc.tensor.matmul(apsA[:, 0:mw],
                                         zT[0:DH, bass.ts(j0, 128)],
                                         wot[0:DH, mo:mo + mw],
                                         start=True, stop=True)
                        nc.tensor.matmul(apsB[:, 0:mw],
                                         zT[DH:128, bass.ts(j1, 128)],
                                         wot[DH:128, mo:mo + mw],
                                         start=True, stop=True)
                        nc.vector.tensor_scalar_mul(
                            ob[:, 0, mo:mo + mw], apsA[:, 0:mw],
                            rc[:, j0:j0 + 1])
                        nc.vector.tensor_scalar_mul(
                            ob[:, 1, mo:mo + mw], apsB[:, 0:mw],
                            rc[:, j1:j1 + 1])
                    nc.scalar.dma_start(
                        out[h, bass.ts(jj, 256), :]
                           .rearrange("(a p) m -> p a m", p=128),
                        ob[:])

            emit_loads(0)
            emit_proj_qk(0)
            emit_proj_v(0)
            emit_vp(0)
            for h in range(HPC):
                nxt, prv = h + 1, h - 1
                if nxt < HPC:
                    emit_loads(nxt)
                acts = {0: [], 1: [], 2: [], 3: []}
                if prv >= 0:
                    acts[0].append(lambda p=prv: emit_outproj(p, (4, 5)))
                    acts[1].append(lambda p=prv: emit_outproj(p, (6, 7)))
                if nxt < HPC:
                    acts[1].append(lambda n=nxt: emit_proj_qk(n))
                    acts[2].append(lambda n=nxt: emit_proj_v(n))
                    acts[3].append(lambda n=nxt: emit_vp(n))
                if debug and h == 0:
                    nc.gpsimd.dma_start(dqT[:], st[0]["qT"][:])
                    nc.gpsimd.dma_start(dkT[:], st[0]["kT"][:])
                    nc.gpsimd.dma_start(dvT[:], st[0]["vT"][:])
                    for i in range(NT):
                        nc.gpsimd.dma_start(dvp[:, i, :], st[0]["vp"][i][:, 0:DH])
                emit_pass(h, (0, 1), {c: acts[c] for c in (0, 1)})
                emit_outproj(h, (0, 1, 2, 3))
                emit_pass(h, (2, 3), {c: acts[c] for c in (2, 3)})
                if debug and h == 0:
                    nc.gpsimd.dma_start(dzT[:], st[0]["zT"][:])
                    nc.gpsimd.dma_start(drc[:], st[0]["rc"][:])
            emit_outproj(HPC - 1, (4, 5, 6, 7))
    nc.compile()
    return nc


_CACHED = None


def _program(debug=False):
    global _CACHED
    if _CACHED is None:
        _CACHED = build_program(debug)
    return _CACHED


def _make_in_maps(inputs):
    xq_f = np.asarray(inputs["normalized_resid_pre_q"], dtype=np.float32)
    xk_f = np.asarray(inputs["normalized_resid_pre_k"], dtype=np.float32)
    xv_f = np.asarray(inputs["normalized_resid_pre_v"], dtype=np.float32)
    WQ = np.asarray(inputs["W_Q"], dtype=np.float32) * WSC
    WK = np.asarray(inputs["W_K"], dtype=np.float32) * WSC
    WV = np.asarray(inputs["W_V"], dtype=np.float32)
    WO = np.asarray(inputs["W_O"], dtype=np.float32)
    bQ = np.asarray(inputs["b_Q"], dtype=np.float32) * WSC
    bK = np.asarray(inputs["b_K"], dtype=np.float32) * WSC
    bV = np.asarray(inputs["b_V"], dtype=np.float32)
    bO = np.asarray(inputs["b_O"], dtype=np.float32)

    def interleave_x(x):  # [DM, S] -> [128, 3, 2, S], m = 256b + 128k + p
        return np.ascontiguousarray(
            x.reshape(3, 2, 128, -1).transpose(2, 0, 1, 3))

    def interleave_w(w):  # [DM, DH] -> [128, 3, 2, DH]
        return np.ascontiguousarray(
            w.reshape(3, 2, 128, DH).transpose(2, 0, 1, 3))

    identh = np.eye(128, dtype=np.float16)
    masku = ((np.arange(128)[:, None] > np.arange(128)[None, :])
             .astype(np.float16) * np.float16(NEG))

    bq2 = np.zeros((H, 128, 1), np.float32)
    bq2[:, 0:DH, 0] = bQ
    bq2[:, DH:128, 0] = bQ
    bk2 = np.zeros((H, 128, 1), np.float32)
    bk2[:, 0:DH, 0] = bK
    bk2[:, DH:128, 0] = bK
    bv2 = np.zeros((H, 128, 1), np.float32)
    bv2[:, 0:DH, 0] = bV
    bv2[:, DH:128, 0] = bV

    in_maps = []
    for c in range(N_CORES):
        b = c % 2
        hg = c // 2
        hs = slice(HPC * hg, HPC * hg + HPC)
        m = {
            "xq": np.stack([interleave_x(
                xq_f[b, :, HPC * hg + j, :].T) for j in range(HPC)]
                ).astype(NP_X),
            "xk": np.stack([interleave_x(
                xk_f[b, :, HPC * hg + j, :].T) for j in range(HPC)]
                ).astype(NP_X),
            "xv": np.ascontiguousarray(
                xv_f[b, :, hs, :].transpose(1, 2, 0)).astype(np.float16),
            "wq": np.stack([interleave_w(WQ[HPC * hg + j])
                            for j in range(HPC)]).astype(NP_X),
            "wk": np.stack([interleave_w(WK[HPC * hg + j])
                            for j in range(HPC)]).astype(NP_X),
            "wv": np.ascontiguousarray(
                WV[hs].reshape(HPC, MT, 128, DH)).astype(np.float16),
            "wo": np.ascontiguousarray(np.concatenate(
                [WO[hs], WO[hs]], axis=1)).astype(np.float16),
            "bq": np.ascontiguousarray(bq2[hs]),
            "bk": np.ascontiguousarray(bk2[hs]),
            "bv": np.ascontiguousarray(bv2[hs]),
            "identh": identh,
            "masku": masku,
        }
        in_maps.append(m)
    return in_maps


def run(inputs, trace=False, debug=False, **kw):
    nc = _program(debug)
    in_maps = _make_in_maps(inputs)
    res = run_bass_kernel_spmd(nc, in_maps, core_ids=list(range(N_CORES)),
                               trace=trace, **kw)
    full = np.zeros((B, S, H, DM), np.float32)
    for c in range(N_CORES):
        b = c % 2
        hg = c // 2
        o = res.results[c]["out"]
        for j in range(HPC):
            full[b, :, HPC * hg + j, :] = o[j]
    bO = np.asarray(inputs["b_O"], dtype=np.float32)
    if np.any(bO):
        full += bO / H
    return full, res


def kernel(**inputs):
    full, _ = run(inputs)
    return full
